# revision 15
# baseline (speedup 1.0000x reference)
"""Trainium2 Bass kernel for nn_DiffeqSolver (two-step Euler MLP-ODE).

Math (per trajectory n, time step i):
    f(y) = tanh(y@W1 + b1)@W2 + b2
    h_i = t_i / 2
    y1_i = y0 + h_i*f(y0)
    y2_i = y1_i + h_i*f(y1_i)
    out[n,i,:] = active[n,i] ? y2_i : 0      (active = any(mask[n,i,:] > 0))
    (t_i == 0 gives y2 == y0 exactly, so the reference's pos-branch is folded.)

Refactor used on device (feature-major, per core with T=1024 trajectories):
    g0  = f(y0)                      (once)
    z_i = W1^T y0^T + (h_i W1)^T g0^T + b1          (PE accumulate, 2 matmuls)
    u_i = tanh(z_i)                                  (ACT)
    y2_i^(chunk) = [Cb;y0]^T_chunk @ rhsD_i + (u_i_chunk)^T @ (h_i W2)
                   where Cb = g0 + b2, rhsD packs [h_i*I64; I64]
    -> y2 lands TRAJ-major in PSUM; one fused DVE op applies the active-mask
       and drains PSUM->SBUF; DMA out contiguous [128 traj, 8 steps x 64 lat].

Sharding: data-parallel over trajectories, 1024 per core x 8 cores.
"""

import numpy as np
from contextlib import ExitStack

import concourse.bass as bass
import concourse.bacc as bacc
import concourse.mybir as mybir
import concourse.tile as tile
import concourse.tile_sem_assignment as _tsa
from concourse.tile import add_dep_helper as _add_dep
from concourse.bass_utils import run_bass_kernel_spmd

# Single SWDGE sem lane: DMA-completion waits on consumers merge into one
# cumulative sem wait, keeping every instruction at <=2 sync waits (the
# walrus codegen limit this kernel was hitting with 8 round-robin lanes).
_tsa.NUM_SWDGE_GLOBAL_SEMS = 1

N_TRAJ, N_TIME, LAT, HID, DIM = 8192, 64, 64, 128, 64
NCORES = 8
T = N_TRAJ // NCORES          # 1024 trajectories per core
NCH = T // 128                # 8 chunks of 128 trajectories
SG = 8                        # steps per group
NG = N_TIME // SG             # 8 step groups
F32 = mybir.dt.float32
I32 = mybir.dt.int32
AF = mybir.ActivationFunctionType
OP = mybir.AluOpType

_cache = {}


def _emit(ctx, tc, nc, fp, ts, mask, W1, b1, W2, b2, out, ident, ipat_full):
    P = 128

    const = ctx.enter_context(tc.tile_pool(name="const", bufs=1))

    W1_sb = const.tile([LAT, HID], F32)
    nc.gpsimd.dma_start(W1_sb[:], W1[:])
    W2_sb = const.tile([HID, LAT], F32)
    nc.gpsimd.dma_start(W2_sb[:], W2[:])
    b1_sb = const.tile([HID, 1], F32)
    nc.gpsimd.dma_start(b1_sb[:], b1[:])
    b2_sb = const.tile([LAT, 1], F32)
    nc.gpsimd.dma_start(b2_sb[:], b2[:])
    id_sb = const.tile([P, P], F32)
    nc.gpsimd.dma_start(id_sb[:], ident[:])
    ip2_sb = const.tile([P, SG * LAT], F32)
    nc.gpsimd.dma_start(ip2_sb[:], ipat_full[:])
    ts_sb = const.tile([1, N_TIME], F32)
    nc.gpsimd.dma_start(ts_sb[:], ts[:])

    ones_sb = const.tile([1, P], F32)
    nc.vector.memset(ones_sb[:], 1.0)
    hrow = const.tile([1, N_TIME], F32)
    nc.vector.tensor_scalar_mul(hrow[:], ts_sb[:], 0.5)

    # Hcol[p, i] = h_i on all 128 partitions (K=1 matmul broadcast).
    # All stage-1 PSUM tiles share one slot (tag s1) so the pool stays at
    # 1 bank and never needs releasing (no released-zone deps downstream).
    s1psum = ctx.enter_context(tc.tile_pool(name="s1psum", bufs=1, space="PSUM"))
    psH = s1psum.tile([P, 512], F32, tag="s1")
    nc.tensor.matmul(psH[:, 0:N_TIME], ones_sb[:], hrow[:], start=True, stop=True)
    Hcol = const.tile([P, N_TIME], F32)
    nc.vector.tensor_copy(Hcol[:], psH[:, 0:N_TIME])

    # rhsD[:, g*512 + s*64 + l'] : rows 0:64 = h_{8g+s}*I64, rows 64:128 = I64.
    rhsD = const.tile([P, NG * SG * LAT], F32)
    for g in range(NG):
        sl = slice(g * SG * LAT, (g + 1) * SG * LAT)
        nc.vector.tensor_mul(
            rhsD[0:LAT, sl].rearrange("p (s l) -> p s l", l=LAT),
            ip2_sb[0:LAT, :].rearrange("p (s l) -> p s l", l=LAT),
            Hcol[0:LAT, g * SG : (g + 1) * SG][:, :, None].broadcast_to(
                [LAT, SG, LAT]
            ),
        )
        nc.vector.tensor_copy(rhsD[LAT:P, sl], ip2_sb[LAT:P, :])

    # ---- active mask: activeF[p, c*64 + i] = 1.0 if any(mask[c*128+p, i, :]) else 0.
    activeF = const.tile([P, NCH * N_TIME], F32)
    MBUFS = 3
    mpool = ctx.enter_context(tc.tile_pool(name="mask", bufs=MBUFS))
    redp = ctx.enter_context(tc.tile_pool(name="red", bufs=2))
    red_insts = []
    for c in range(NCH):
        if c >= MBUFS:
            # Pool-engine nop carrying the DVE wait for the mt slot being
            # reused, so the DMA itself keeps a single (queue) sync wait.
            nop = nc.gpsimd.engine_nop()
            _add_dep(nop.ins, red_insts[c - MBUFS].ins, sync=True,
                     reason="mask slot reuse pre-wait")
        mt = mpool.tile([P, N_TIME * DIM], I32, tag="mt")
        nc.gpsimd.dma_start(mt[:], mask[c * P : (c + 1) * P, :])
        msum = redp.tile([P, N_TIME], I32, tag="msum")
        red_insts.append(nc.vector.tensor_reduce(
            msum[:],
            mt[:].rearrange("p (t d) -> p t d", d=DIM),
            axis=mybir.AxisListType.X,
            op=OP.max,
        ))
        nc.vector.tensor_scalar(
            activeF[:, c * N_TIME : (c + 1) * N_TIME],
            msum[:],
            0,
            None,
            op0=OP.is_gt,
        )

    # ---- stage 1: y0^T, g0 = f(y0), Cb = g0 + b2 (feature-major).
    # Y0T/G0T: [64, T] z-matmul rhs operands (base partition 0).
    # CY rows 0:64 = Cb^T, rows 64:128 = y0^T     (y-matmul lhsT stack)
    Y0T = const.tile([LAT, T], F32)
    G0T = const.tile([LAT, T], F32)
    CY = const.tile([P, T], F32)
    y0p = ctx.enter_context(tc.tile_pool(name="y0p", bufs=NCH))
    for c in range(NCH):
        # yt2 = [fp_chunk | fp_chunk]; its PE transpose lands y0^T twice,
        # at partitions 0:64 (-> Y0T) and 64:128 (-> CY bottom, base-64
        # aligned so no partition-shifting copy is needed).
        yt2 = y0p.tile([P, 2 * LAT], F32, tag="yt2")
        nc.gpsimd.dma_start(yt2[:, 0:LAT], fp[c * P : (c + 1) * P, :])
        nc.gpsimd.dma_start(yt2[:, LAT : 2 * LAT], fp[c * P : (c + 1) * P, :])
        psT = s1psum.tile([P, 512], F32, tag="s1")
        nc.tensor.transpose(psT[:, 0:P], yt2[:], id_sb[:])
        nc.vector.tensor_copy(Y0T[:, c * P : (c + 1) * P], psT[0:LAT, 0:P])
        nc.vector.tensor_copy(CY[LAT:P, c * P : (c + 1) * P], psT[LAT:P, 0:P])

    b2x2 = const.tile([LAT, 1], F32)
    nc.vector.tensor_scalar_mul(b2x2[:], b2_sb[:], 2.0)

    for hlf in range(2):
        sl = slice(hlf * 512, (hlf + 1) * 512)
        psZ0 = s1psum.tile([P, 512], F32, tag="s1")
        nc.tensor.matmul(psZ0[:], W1_sb[:], Y0T[:, sl], start=True, stop=True)
        u0 = y0p.tile([P, 512], F32, tag="u0")
        nc.scalar.activation(u0[:], psZ0[:], AF.Tanh, bias=b1_sb[:, 0:1])
        psG0 = s1psum.tile([LAT, 512], F32, tag="s1")
        nc.tensor.matmul(psG0[:], W2_sb[:], u0[:], start=True, stop=True)
        nc.scalar.activation(G0T[:, sl], psG0[:], AF.Identity, bias=b2_sb[:, 0:1])
        nc.scalar.activation(CY[0:LAT, sl], psG0[:], AF.Identity, bias=b2x2[:, 0:1])

    # ---- main loop over step groups.
    wpool = ctx.enter_context(tc.tile_pool(name="wpool", bufs=2 * SG + 2))
    zpool = ctx.enter_context(tc.tile_pool(name="zpool", bufs=2, space="PSUM"))
    upool = ctx.enter_context(tc.tile_pool(name="upool", bufs=SG + 2))
    ypool = ctx.enter_context(tc.tile_pool(name="ypool", bufs=3, space="PSUM"))
    opool = ctx.enter_context(tc.tile_pool(name="opool", bufs=3))

    for g in range(NG):
        uts = []
        w2s = []
        for s in range(SG):
            i = g * SG + s
            w1s = wpool.tile([LAT, HID], F32, tag="w1s")
            nc.vector.tensor_scalar_mul(w1s[:], W1_sb[:], Hcol[0:LAT, i : i + 1])
            w2si = wpool.tile([HID, LAT], F32, tag="w2s")
            nc.vector.tensor_scalar_mul(w2si[:], W2_sb[:], Hcol[:, i : i + 1])
            w2s.append(w2si)

            psZ = zpool.tile([P, 1024], F32, tag="psZ")
            for hlf in range(2):
                sl = slice(hlf * 512, (hlf + 1) * 512)
                nc.tensor.matmul(
                    psZ[:, sl], W1_sb[:], Y0T[:, sl], start=True, stop=False
                )
                nc.tensor.matmul(
                    psZ[:, sl], w1s[:], G0T[:, sl], start=False, stop=True
                )
            ut = upool.tile([P, 1024], F32, tag="ut")
            nc.scalar.activation(ut[:], psZ[:], AF.Tanh, bias=b1_sb[:, 0:1])
            uts.append(ut)

        for c in range(NCH):
            csl = slice(c * P, (c + 1) * P)
            psY = ypool.tile([P, SG * LAT], F32, tag="psY")
            nc.tensor.matmul(
                psY[:],
                CY[:, csl],
                rhsD[:, g * SG * LAT : (g + 1) * SG * LAT],
                start=True,
                stop=False,
            )
            for s in range(SG):
                nc.tensor.matmul(
                    psY[:, s * LAT : (s + 1) * LAT],
                    uts[s][:, csl],
                    w2s[s][:],
                    start=False,
                    stop=(s == SG - 1),
                    skip_group_check=True,
                )
            ot = opool.tile([P, SG * LAT], F32, tag="ot")
            drain = nc.vector.scalar_tensor_tensor(
                ot[:].rearrange("p (s l) -> p s l", l=LAT),
                psY[:].rearrange("p (s l) -> p s l", l=LAT),
                1.0,
                activeF[:, c * N_TIME + g * SG : c * N_TIME + (g + 1) * SG][
                    :, :, None
                ].broadcast_to([P, SG, LAT]),
                op0=OP.bypass,
                op1=OP.mult,
            )
            nop = nc.gpsimd.engine_nop()
            _add_dep(nop.ins, drain.ins, sync=True, reason="out pre-wait")
            nc.gpsimd.dma_start(
                out[c * P : (c + 1) * P, g * SG * LAT : (g + 1) * SG * LAT], ot[:]
            )


def _build():
    if "nc" in _cache:
        return _cache["nc"]
    nc = bacc.Bacc("TRN2", target_bir_lowering=False, debug=False)
    fp = nc.dram_tensor("fp", [T, LAT], F32, kind="ExternalInput")
    ts = nc.dram_tensor("ts", [1, N_TIME], F32, kind="ExternalInput")
    mask = nc.dram_tensor("mask", [T, N_TIME * DIM], I32, kind="ExternalInput")
    W1 = nc.dram_tensor("W1", [LAT, HID], F32, kind="ExternalInput")
    b1 = nc.dram_tensor("b1", [HID, 1], F32, kind="ExternalInput")
    W2 = nc.dram_tensor("W2", [HID, LAT], F32, kind="ExternalInput")
    b2 = nc.dram_tensor("b2", [LAT, 1], F32, kind="ExternalInput")
    out = nc.dram_tensor("out", [T, N_TIME * LAT], F32, kind="ExternalOutput")

    ident = nc.inline_tensor(np.eye(128, dtype=np.float32), name="ident")
    ipat_full = nc.inline_tensor(
        np.tile(np.eye(LAT, dtype=np.float32), (2, SG)), name="ipat_full"
    )

    with tile.TileContext(nc) as tc:
        with ExitStack() as ctx:
            _emit(ctx, tc, nc, fp, ts, mask, W1, b1, W2, b2, out, ident, ipat_full)
    _strip_same_engine_waits(nc)
    nc.compile()
    _cache["nc"] = nc
    return nc


def _strip_same_engine_waits(nc):
    """Drop sem waits that target the instruction's own engine proc sem.

    Each engine executes its instruction stream in order and its proc sem is
    only incremented by its own completed instructions, so a wait on your own
    engine's sem is satisfied by construction. Tile emits these conservatively
    (it doesn't track transitive same-proc knowledge); walrus codegen caps
    instructions at 2 sync waits, so the redundant ones must go.
    """
    eng_prefix = {
        "PE": "PE_",
        "DVE": "DVE_",
        "Activation": "Activation_",
        "SP": "SP_",
        "Pool": "Pool_",
    }
    for fn in nc.m.functions:
        for blk in fn.blocks:
            for inst in blk.instructions:
                si = getattr(inst, "sync_info", None)
                if si is None or not si.on_wait:
                    continue
                eng = getattr(inst, "engine", None)
                pref = eng_prefix.get(getattr(eng, "value", None) or str(eng), None)
                if pref is None:
                    continue
                kept = [
                    w
                    for w in si.on_wait
                    if not (getattr(w, "ant_name", "") or "").startswith(pref)
                ]
                if len(kept) != len(si.on_wait):
                    si.on_wait = kept


def kernel(first_point, time_steps, mask, W1, b1, W2, b2, trace=False, **trace_kw):
    first_point = np.asarray(first_point)
    time_steps = np.asarray(time_steps)
    mask = np.asarray(mask)
    W1a = np.ascontiguousarray(np.asarray(W1), dtype=np.float32)
    b1a = np.ascontiguousarray(np.asarray(b1), dtype=np.float32).reshape(HID, 1)
    W2a = np.ascontiguousarray(np.asarray(W2), dtype=np.float32)
    b2a = np.ascontiguousarray(np.asarray(b2), dtype=np.float32).reshape(LAT, 1)
    tsa = np.ascontiguousarray(time_steps, dtype=np.float32).reshape(1, N_TIME)

    fp_full = np.ascontiguousarray(first_point[0], dtype=np.float32)  # [8192, 64]
    mask_full = np.ascontiguousarray(mask, dtype=np.int32).reshape(
        N_TRAJ, N_TIME * DIM
    )

    nc = _build()
    in_maps = []
    for c in range(NCORES):
        sl = slice(c * T, (c + 1) * T)
        in_maps.append(
            {
                "fp": np.ascontiguousarray(fp_full[sl]),
                "ts": tsa,
                "mask": np.ascontiguousarray(mask_full[sl]),
                "W1": W1a,
                "b1": b1a,
                "W2": W2a,
                "b2": b2a,
            }
        )

    res = run_bass_kernel_spmd(
        nc, in_maps, core_ids=list(range(NCORES)), trace=trace, **trace_kw
    )
    outs = [r["out"].reshape(T, N_TIME, 1, LAT) for r in res.results]
    full = np.concatenate(outs, axis=0)
    if trace:
        kernel.last_result = res
    return full


# revision 16
# speedup vs baseline: 10819.4226x; 10819.4226x over previous
"""Trainium2 Bass kernel for nn_DiffeqSolver (two-step Euler MLP-ODE).

Math (per trajectory n, time step i):
    f(y) = tanh(y@W1 + b1)@W2 + b2
    h_i = t_i / 2
    y1_i = y0 + h_i*f(y0)
    y2_i = y1_i + h_i*f(y1_i)
    out[n,i,:] = active[n,i] ? y2_i : 0      (active = any(mask[n,i,:] > 0))
    (t_i == 0 gives y2 == y0 exactly, so the reference's pos-branch is folded.)

Refactor used on device (feature-major, per core with T=1024 trajectories):
    g0  = f(y0)                      (once)
    z_i = W1^T y0^T + (h_i W1)^T g0^T + b1          (PE accumulate, 2 matmuls)
    u_i = tanh(z_i)                                  (ACT)
    y2_i^(chunk) = [Cb;y0]^T_chunk @ rhsD_i + (u_i_chunk)^T @ (h_i W2)
                   where Cb = g0 + b2, rhsD packs [h_i*I64; I64]
    -> y2 lands TRAJ-major in PSUM; one fused DVE op applies the active-mask
       and drains PSUM->SBUF; DMA out contiguous [128 traj, 8 steps x 64 lat].

Sharding: data-parallel over trajectories, 1024 per core x 8 cores.
"""

import numpy as np
from contextlib import ExitStack

import concourse.bass as bass
import concourse.bacc as bacc
import concourse.mybir as mybir
import concourse.tile as tile
import concourse.tile_sem_assignment as _tsa
from concourse.tile import add_dep_helper as _add_dep
from concourse.bass_utils import run_bass_kernel_spmd

# NOTE: excess sync waits (>1 per instruction) are legal here because
# Bacc.compile()'s generate_event_semaphores pass splits them; the nop
# pre-waits below just keep DMA instructions lean.

N_TRAJ, N_TIME, LAT, HID, DIM = 8192, 64, 64, 128, 64
NCORES = 8
T = N_TRAJ // NCORES          # 1024 trajectories per core
NCH = T // 128                # 8 chunks of 128 trajectories
SG = 8                        # steps per group
NG = N_TIME // SG             # 8 step groups
F32 = mybir.dt.float32
I32 = mybir.dt.int32
AF = mybir.ActivationFunctionType
OP = mybir.AluOpType

_cache = {}


def _emit(ctx, tc, nc, fp, ts, mask, W1, b1, W2, b2, out, ident, ipat_full):
    P = 128

    const = ctx.enter_context(tc.tile_pool(name="const", bufs=1))

    W1_sb = const.tile([LAT, HID], F32)
    nc.gpsimd.dma_start(W1_sb[:], W1[:])
    W2_sb = const.tile([HID, LAT], F32)
    nc.gpsimd.dma_start(W2_sb[:], W2[:])
    b1_sb = const.tile([HID, 1], F32)
    nc.gpsimd.dma_start(b1_sb[:], b1[:])
    b2_sb = const.tile([LAT, 1], F32)
    nc.gpsimd.dma_start(b2_sb[:], b2[:])
    id_sb = const.tile([P, P], F32)
    nc.gpsimd.dma_start(id_sb[:], ident[:])
    ip2_sb = const.tile([P, SG * LAT], F32)
    nc.gpsimd.dma_start(ip2_sb[:], ipat_full[:])
    ts_sb = const.tile([1, N_TIME], F32)
    nc.gpsimd.dma_start(ts_sb[:], ts[:])

    ones_sb = const.tile([1, P], F32)
    nc.vector.memset(ones_sb[:], 1.0)
    hrow = const.tile([1, N_TIME], F32)
    nc.vector.tensor_scalar_mul(hrow[:], ts_sb[:], 0.5)

    # Hcol[p, i] = h_i on all 128 partitions (K=1 matmul broadcast).
    # All stage-1 PSUM tiles share one slot (tag s1) so the pool stays at
    # 1 bank and never needs releasing (no released-zone deps downstream).
    s1psum = ctx.enter_context(tc.tile_pool(name="s1psum", bufs=1, space="PSUM"))
    psH = s1psum.tile([P, 512], F32, tag="s1")
    nc.tensor.matmul(psH[:, 0:N_TIME], ones_sb[:], hrow[:], start=True, stop=True)
    Hcol = const.tile([P, N_TIME], F32)
    nc.vector.tensor_copy(Hcol[:], psH[:, 0:N_TIME])

    # rhsD[:, g*512 + s*64 + l'] : rows 0:64 = h_{8g+s}*I64, rows 64:128 = I64.
    rhsD = const.tile([P, NG * SG * LAT], F32)
    for g in range(NG):
        sl = slice(g * SG * LAT, (g + 1) * SG * LAT)
        nc.vector.tensor_mul(
            rhsD[0:LAT, sl].rearrange("p (s l) -> p s l", l=LAT),
            ip2_sb[0:LAT, :].rearrange("p (s l) -> p s l", l=LAT),
            Hcol[0:LAT, g * SG : (g + 1) * SG][:, :, None].broadcast_to(
                [LAT, SG, LAT]
            ),
        )
        nc.vector.tensor_copy(rhsD[LAT:P, sl], ip2_sb[LAT:P, :])

    # ---- active mask: activeF[p, c*64 + i] = 1.0 if any(mask[c*128+p, i, :]) else 0.
    activeF = const.tile([P, NCH * N_TIME], F32)
    MBUFS = 3
    mpool = ctx.enter_context(tc.tile_pool(name="mask", bufs=MBUFS))
    redp = ctx.enter_context(tc.tile_pool(name="red", bufs=2))
    red_insts = []
    for c in range(NCH):
        if c >= MBUFS:
            # Pool-engine nop carrying the DVE wait for the mt slot being
            # reused, so the DMA itself keeps a single (queue) sync wait.
            nop = nc.gpsimd.engine_nop()
            _add_dep(nop.ins, red_insts[c - MBUFS].ins, sync=True,
                     reason="mask slot reuse pre-wait")
        mt = mpool.tile([P, N_TIME * DIM], I32, tag="mt")
        nc.gpsimd.dma_start(mt[:], mask[c * P : (c + 1) * P, :])
        msum = redp.tile([P, N_TIME], I32, tag="msum")
        red_insts.append(nc.vector.tensor_reduce(
            msum[:],
            mt[:].rearrange("p (t d) -> p t d", d=DIM),
            axis=mybir.AxisListType.X,
            op=OP.max,
        ))
        nc.vector.tensor_scalar(
            activeF[:, c * N_TIME : (c + 1) * N_TIME],
            msum[:],
            0,
            None,
            op0=OP.is_gt,
        )

    # ---- stage 1: y0^T, g0 = f(y0), Cb = g0 + b2 (feature-major).
    # Y0T/G0T: [64, T] z-matmul rhs operands (base partition 0).
    # CY rows 0:64 = Cb^T, rows 64:128 = y0^T     (y-matmul lhsT stack)
    Y0T = const.tile([LAT, T], F32)
    G0T = const.tile([LAT, T], F32)
    CY = const.tile([P, T], F32)
    y0p = ctx.enter_context(tc.tile_pool(name="y0p", bufs=NCH))
    for c in range(NCH):
        # yt2 = [fp_chunk | fp_chunk]; its PE transpose lands y0^T twice,
        # at partitions 0:64 (-> Y0T) and 64:128 (-> CY bottom, base-64
        # aligned so no partition-shifting copy is needed).
        yt2 = y0p.tile([P, 2 * LAT], F32, tag="yt2")
        nc.gpsimd.dma_start(yt2[:, 0:LAT], fp[c * P : (c + 1) * P, :])
        nc.gpsimd.dma_start(yt2[:, LAT : 2 * LAT], fp[c * P : (c + 1) * P, :])
        psT = s1psum.tile([P, 512], F32, tag="s1")
        nc.tensor.transpose(psT[:, 0:P], yt2[:], id_sb[:])
        nc.vector.tensor_copy(Y0T[:, c * P : (c + 1) * P], psT[0:LAT, 0:P])
        nc.vector.tensor_copy(CY[LAT:P, c * P : (c + 1) * P], psT[LAT:P, 0:P])

    b2x2 = const.tile([LAT, 1], F32)
    nc.vector.tensor_scalar_mul(b2x2[:], b2_sb[:], 2.0)

    for hlf in range(2):
        sl = slice(hlf * 512, (hlf + 1) * 512)
        psZ0 = s1psum.tile([P, 512], F32, tag="s1")
        nc.tensor.matmul(psZ0[:], W1_sb[:], Y0T[:, sl], start=True, stop=True)
        u0 = y0p.tile([P, 512], F32, tag="u0")
        nc.scalar.activation(u0[:], psZ0[:], AF.Tanh, bias=b1_sb[:, 0:1])
        psG0 = s1psum.tile([LAT, 512], F32, tag="s1")
        nc.tensor.matmul(psG0[:], W2_sb[:], u0[:], start=True, stop=True)
        nc.scalar.activation(G0T[:, sl], psG0[:], AF.Identity, bias=b2_sb[:, 0:1])
        nc.scalar.activation(CY[0:LAT, sl], psG0[:], AF.Identity, bias=b2x2[:, 0:1])

    # ---- main loop over step groups.
    wpool = ctx.enter_context(tc.tile_pool(name="wpool", bufs=2 * SG + 2))
    zpool = ctx.enter_context(tc.tile_pool(name="zpool", bufs=2, space="PSUM"))
    upool = ctx.enter_context(tc.tile_pool(name="upool", bufs=SG + 2))
    ypool = ctx.enter_context(tc.tile_pool(name="ypool", bufs=3, space="PSUM"))
    opool = ctx.enter_context(tc.tile_pool(name="opool", bufs=3))

    for g in range(NG):
        uts = []
        w2s = []
        for s in range(SG):
            i = g * SG + s
            w1s = wpool.tile([LAT, HID], F32, tag="w1s")
            nc.vector.tensor_scalar_mul(w1s[:], W1_sb[:], Hcol[0:LAT, i : i + 1])
            w2si = wpool.tile([HID, LAT], F32, tag="w2s")
            nc.vector.tensor_scalar_mul(w2si[:], W2_sb[:], Hcol[:, i : i + 1])
            w2s.append(w2si)

            psZ = zpool.tile([P, 1024], F32, tag="psZ")
            for hlf in range(2):
                sl = slice(hlf * 512, (hlf + 1) * 512)
                nc.tensor.matmul(
                    psZ[:, sl], W1_sb[:], Y0T[:, sl], start=True, stop=False
                )
                nc.tensor.matmul(
                    psZ[:, sl], w1s[:], G0T[:, sl], start=False, stop=True
                )
            ut = upool.tile([P, 1024], F32, tag="ut")
            nc.scalar.activation(ut[:], psZ[:], AF.Tanh, bias=b1_sb[:, 0:1])
            uts.append(ut)

        for c in range(NCH):
            csl = slice(c * P, (c + 1) * P)
            psY = ypool.tile([P, SG * LAT], F32, tag="psY")
            nc.tensor.matmul(
                psY[:],
                CY[:, csl],
                rhsD[:, g * SG * LAT : (g + 1) * SG * LAT],
                start=True,
                stop=False,
            )
            for s in range(SG):
                nc.tensor.matmul(
                    psY[:, s * LAT : (s + 1) * LAT],
                    uts[s][:, csl],
                    w2s[s][:],
                    start=False,
                    stop=(s == SG - 1),
                    skip_group_check=True,
                )
            ot = opool.tile([P, SG * LAT], F32, tag="ot")
            drain = nc.vector.scalar_tensor_tensor(
                ot[:].rearrange("p (s l) -> p s l", l=LAT),
                psY[:].rearrange("p (s l) -> p s l", l=LAT),
                1.0,
                activeF[:, c * N_TIME + g * SG : c * N_TIME + (g + 1) * SG][
                    :, :, None
                ].broadcast_to([P, SG, LAT]),
                op0=OP.bypass,
                op1=OP.mult,
            )
            nop = nc.gpsimd.engine_nop()
            _add_dep(nop.ins, drain.ins, sync=True, reason="out pre-wait")
            nc.gpsimd.dma_start(
                out[c * P : (c + 1) * P, g * SG * LAT : (g + 1) * SG * LAT], ot[:]
            )


def _build():
    if "nc" in _cache:
        return _cache["nc"]
    nc = bacc.Bacc("TRN2", target_bir_lowering=False, debug=False)
    fp = nc.dram_tensor("fp", [T, LAT], F32, kind="ExternalInput")
    ts = nc.dram_tensor("ts", [1, N_TIME], F32, kind="ExternalInput")
    mask = nc.dram_tensor("mask", [T, N_TIME * DIM], I32, kind="ExternalInput")
    W1 = nc.dram_tensor("W1", [LAT, HID], F32, kind="ExternalInput")
    b1 = nc.dram_tensor("b1", [HID, 1], F32, kind="ExternalInput")
    W2 = nc.dram_tensor("W2", [HID, LAT], F32, kind="ExternalInput")
    b2 = nc.dram_tensor("b2", [LAT, 1], F32, kind="ExternalInput")
    out = nc.dram_tensor("out", [T, N_TIME * LAT], F32, kind="ExternalOutput")

    ident = nc.inline_tensor(np.eye(128, dtype=np.float32), name="ident")
    ipat_full = nc.inline_tensor(
        np.tile(np.eye(LAT, dtype=np.float32), (2, SG)), name="ipat_full"
    )

    with tile.TileContext(nc) as tc:
        with ExitStack() as ctx:
            _emit(ctx, tc, nc, fp, ts, mask, W1, b1, W2, b2, out, ident, ipat_full)
    _strip_same_engine_waits(nc)
    nc.compile()
    _cache["nc"] = nc
    return nc


def _strip_same_engine_waits(nc):
    """Drop sem waits that target the instruction's own engine proc sem.

    Each engine executes its instruction stream in order and its proc sem is
    only incremented by its own completed instructions, so a wait on your own
    engine's sem is satisfied by construction. Tile emits these conservatively
    (it doesn't track transitive same-proc knowledge); walrus codegen caps
    instructions at 2 sync waits, so the redundant ones must go.
    """
    eng_prefix = {
        "PE": "PE_",
        "DVE": "DVE_",
        "Activation": "Activation_",
        "SP": "SP_",
        "Pool": "Pool_",
    }
    for fn in nc.m.functions:
        for blk in fn.blocks:
            for inst in blk.instructions:
                si = getattr(inst, "sync_info", None)
                if si is None or not si.on_wait:
                    continue
                eng = getattr(inst, "engine", None)
                pref = eng_prefix.get(getattr(eng, "value", None) or str(eng), None)
                if pref is None:
                    continue
                kept = [
                    w
                    for w in si.on_wait
                    if not (getattr(w, "ant_name", "") or "").startswith(pref)
                ]
                if len(kept) != len(si.on_wait):
                    si.on_wait = kept


def kernel(first_point, time_steps, mask, W1, b1, W2, b2, trace=False, **trace_kw):
    first_point = np.asarray(first_point)
    time_steps = np.asarray(time_steps)
    mask = np.asarray(mask)
    W1a = np.ascontiguousarray(np.asarray(W1), dtype=np.float32)
    b1a = np.ascontiguousarray(np.asarray(b1), dtype=np.float32).reshape(HID, 1)
    W2a = np.ascontiguousarray(np.asarray(W2), dtype=np.float32)
    b2a = np.ascontiguousarray(np.asarray(b2), dtype=np.float32).reshape(LAT, 1)
    tsa = np.ascontiguousarray(time_steps, dtype=np.float32).reshape(1, N_TIME)

    fp_full = np.ascontiguousarray(first_point[0], dtype=np.float32)  # [8192, 64]
    mask_full = np.ascontiguousarray(mask, dtype=np.int32).reshape(
        N_TRAJ, N_TIME * DIM
    )

    nc = _build()
    in_maps = []
    for c in range(NCORES):
        sl = slice(c * T, (c + 1) * T)
        in_maps.append(
            {
                "fp": np.ascontiguousarray(fp_full[sl]),
                "ts": tsa,
                "mask": np.ascontiguousarray(mask_full[sl]),
                "W1": W1a,
                "b1": b1a,
                "W2": W2a,
                "b2": b2a,
            }
        )

    res = run_bass_kernel_spmd(
        nc, in_maps, core_ids=list(range(NCORES)), trace=trace, **trace_kw
    )
    outs = [r["out"].reshape(T, N_TIME, 1, LAT) for r in res.results]
    full = np.concatenate(outs, axis=0)
    if trace:
        kernel.last_result = res
    return full
